# revision 8
# baseline (speedup 1.0000x reference)
"""MoE block (RMSNorm + top-4 router + 32-expert GLU FFN) on 8 TRN2 NeuronCores.

Expert-parallel: core c owns experts [4c, 4c+4). Each core redundantly
computes the RMSNorm + router over all 32 experts (fp16 gate matmul, f32
top-k/softmax), then runs a dense masked GLU FFN over all 64 tokens for its
own 4 experts with fp8(e4m3) weights and DoubleRow (2x-rate) matmuls.

Memory-bound problem: the weight stream (w1/w2) dominates. Weights are
host-cast to fp8 e4m3 with a power-of-2 scale (S=256 centers the 0.02-std
values in the e4m3 normal range) and host-pre-tiled to (128, e, c, i) so
each per-expert DMA is one fully-contiguous 6400B/3200B-per-partition
transfer on the sync HWDGE ring. The w1 columns are host-permuted to
[glu 0:512 | lin 0:512 | glu 512:640 | lin 512:640] so every h-psum tile is
a contiguous column slice and every matmul (incl. the paired-chunk
DoubleRow ops) uses a simple 3-dim access pattern.

Scale bookkeeping: h' = S1*(t@w1+b1) in psum. glu: p = Silu(h'*BETA/S1) =
BETA*glu_act. lin: l3 = A*(clip+1) via one ACT op with per-token AP
scale/bias (A = routing weight; folds the expert mix into hact). hact =
p*l3 = BETA*A*hact_true, cast fp8; mm2 with w2*S2 accumulates
S2*BETA*sum_e A_e*(hact@w2) for all 4 experts plus the A-weighted b2 bias
(host-scaled by S2*BETA) into one psum group; final copy descales by
1/(S2*BETA). Host sums the 8 per-core partials and adds the residual.
"""

import sys
import types

sys.path.insert(0, "/opt/trn_rl_repo")

import numpy as np

D = 640
I = 640
E = 32
T = 64
K = 4
EPS = 1e-5
LIMIT = 7.0
BETA = 1.702
NCORES = 8
EPC = E // NCORES          # experts per core
NCH = D // 128             # 5 contraction chunks of 128
S1 = 256.0                 # host scale on w1/b1
S2 = 256.0                 # host scale on w2

TRACE = False
PROF_DIR = None
LAST_EXEC_NS = None

_NC = None


def _ensure_ntff_hook():
    """boot() skips NTFF hook registration (image antenv lacks axon_hooks);
    provide the module so bass_utils can profile when TRACE=True."""
    if "antenv.axon_hooks" in sys.modules:
        return
    try:
        from trn_agent_boot.trn_boot import _ntff_profile_via_ctypes
        hook = _ntff_profile_via_ctypes("/opt/axon/libaxon_pjrt.so")
    except Exception:
        hook = None
    m = types.ModuleType("antenv.axon_hooks")
    m.get_axon_ntff_profile_hook = lambda: hook
    m.set_axon_ntff_profile_hook = lambda h: None
    sys.modules["antenv.axon_hooks"] = m


# h psum tiles after the host column permutation: contiguous slices.
# (tag, col offset, width, bufs). hsm = [glu 512:640 | lin 512:640].
H_TILES = [("hgb", 0, 512, 2), ("hlb", 512, 512, 2), ("hsm", 1024, 256, 1)]


def _build():
    import concourse.bass as bass
    import concourse.bacc as bacc
    import concourse.tile as tile
    from concourse import mybir
    from concourse.masks import make_identity

    f32 = mybir.dt.float32
    f16 = mybir.dt.float16
    f8 = mybir.dt.float8e4
    AF = mybir.ActivationFunctionType
    OP = mybir.AluOpType
    DR = mybir.MatmulPerfMode.DoubleRow

    nc = bacc.Bacc("TRN2", target_bir_lowering=False, debug=False,
                   num_devices=NCORES)
    dx = nc.dram_tensor("x", (D, T), f32, kind="ExternalInput")
    dnw = nc.dram_tensor("norm_w", (D,), f32, kind="ExternalInput")
    dgw = nc.dram_tensor("gate_w", (E, D), f32, kind="ExternalInput")
    dgb = nc.dram_tensor("gate_b", (E,), f32, kind="ExternalInput")
    dw1 = nc.dram_tensor("w1", (128, EPC, NCH, 2 * I), f8,
                         kind="ExternalInput")
    db1 = nc.dram_tensor("b1", (EPC, 2 * I), f16, kind="ExternalInput")
    dw2 = nc.dram_tensor("w2", (128, EPC, NCH, D), f8, kind="ExternalInput")
    db2 = nc.dram_tensor("b2", (EPC, D), f16, kind="ExternalInput")
    dout = nc.dram_tensor("out", (T, D), f32, kind="ExternalOutput")

    with tile.TileContext(nc) as tc:
        with (
            tc.tile_pool(name="consts", bufs=1) as consts,
            tc.tile_pool(name="small", bufs=2) as small,
            tc.tile_pool(name="hpool", bufs=2) as hpool,
            tc.tile_pool(name="ps_o", bufs=1, space="PSUM") as ps_o,
        ):
            # ---- sync HWDGE ring order: router inputs first, then the
            # expert weight stream; w2_3 before w1_3 shortens the tail ----
            x_t = consts.tile([128, NCH, T], f32)
            nc.sync.dma_start(out=x_t,
                              in_=dx.ap().rearrange("(c p) t -> p c t", p=128))
            gwn = consts.tile([E, D], f32)
            nc.sync.dma_start(out=gwn, in_=dgw.ap())
            w1_tiles = [consts.tile([128, NCH, 2 * I], f8, tag=f"w1_{e}",
                                    name=f"w1t{e}") for e in range(EPC)]
            w2_tiles = [consts.tile([128, NCH, D], f8, tag=f"w2_{e}",
                                    name=f"w2t{e}") for e in range(EPC)]
            dma_order = []
            for e in range(EPC - 1):
                dma_order += [(w1_tiles[e], dw1, e), (w2_tiles[e], dw2, e)]
            dma_order += [(w2_tiles[EPC - 1], dw2, EPC - 1),
                          (w1_tiles[EPC - 1], dw1, EPC - 1)]
            for (t_, d_, e) in dma_order:
                nc.sync.dma_start(out=t_, in_=d_.ap()[:, e])

            b1_sb = consts.tile([1, EPC * 2 * I], f16)
            nc.gpsimd.dma_start(out=b1_sb,
                                in_=db1.ap().rearrange("e i -> (e i)")[None, :])
            nw_t = consts.tile([128, NCH], f32)
            nc.gpsimd.dma_start(out=nw_t,
                                in_=dnw.ap().rearrange("(c p) -> p c", p=128))
            gb_b = consts.tile([T, E], f32)
            gb_base = dgb.ap()
            nc.gpsimd.dma_start(
                out=gb_b,
                in_=bass.AP(tensor=gb_base.tensor, offset=0,
                            ap=[[0, T], [1, E]]))
            b2_t = consts.tile([EPC, D], f16)
            nc.gpsimd.dma_start(out=b2_t, in_=db2.ap())

            ones128 = consts.tile([128, 128], f32)
            nc.vector.memset(ones128, 1.0)
            ones_hf = consts.tile([1, T], f16)
            nc.vector.memset(ones_hf, 1.0)
            eps_t = consts.tile([128, 1], f32)
            nc.vector.memset(eps_t, EPS)
            id64 = consts.tile([T, T], f32)
            make_identity(nc, id64)
            id_hf = consts.tile([T, T], f16)
            make_identity(nc, id_hf)
            # touch every ACT function once so its table loads during the
            # initial DMA wait instead of inside the critical path
            for fn in (AF.Sqrt, AF.Exp, AF.Sigmoid, AF.Copy, AF.Identity):
                dmy = consts.tile([1, 1], f32, tag=f"dmy{fn}")
                nc.scalar.activation(dmy, eps_t[0:1, :], fn)

            with tc.tile_pool(name="ps_misc", bufs=2, space="PSUM") as ps_misc:
                # ---- RMSNorm (x is (D, T); D on partitions) ----
                xx = small.tile([128, NCH, T], f32, tag="xx")
                nc.vector.tensor_mul(xx, x_t, x_t)
                ps_ss = ps_misc.tile([128, T], f32, tag="misc")
                for c in range(NCH):
                    nc.tensor.matmul(ps_ss, ones128, xx[:, c, :],
                                     start=(c == 0), stop=(c == NCH - 1))
                sq = small.tile([128, T], f32, tag="sq")
                nc.scalar.activation(sq, ps_ss, AF.Sqrt, bias=eps_t,
                                     scale=1.0 / D)
                rstd = small.tile([128, T], f32, tag="rstd")
                nc.vector.reciprocal(rstd, sq)
                normed_hf = consts.tile([128, NCH, T], f16)
                normed_f8 = consts.tile([128, NCH, T], f8)
                for c in range(NCH):
                    xn = small.tile([128, T], f32, tag="xn")
                    nc.vector.tensor_scalar_mul(xn, x_t[:, c, :],
                                                nw_t[:, c:c + 1])
                    nc.vector.tensor_mul(normed_hf[:, c, :], xn, rstd)
                    nc.vector.tensor_mul(normed_f8[:, c, :], xn, rstd)

                # gate_w.T (D on partitions) via PE transpose, cast to fp16
                gwT = consts.tile([128, NCH, E], f16)
                for c in range(NCH):
                    ps_t = ps_misc.tile([128, E], f32, tag="misc")
                    nc.tensor.transpose(ps_t, gwn[:, 128 * c:128 * (c + 1)],
                                        id64[0:E, 0:E])
                    nc.scalar.copy(gwT[:, c, :], ps_t)

                # ---- router: gate, top-4, softmax, routing matrix A ----
                ps_g = ps_misc.tile([T, E], f32, tag="misc")
                for c in range(NCH):
                    nc.tensor.matmul(ps_g, normed_hf[:, c, :], gwT[:, c, :],
                                     start=(c == 0), stop=(c == NCH - 1))
                g_sb = small.tile([T, E], f32, tag="g")
                nc.vector.tensor_add(g_sb, ps_g, gb_b)

            m8 = small.tile([T, 8], f32, tag="m8")
            nc.vector.max(m8, g_sb)
            negm = small.tile([T, 1], f32, tag="negm")
            nc.scalar.mul(negm, m8[:, 0:1], -1.0)
            s4 = small.tile([T, K], f32, tag="s4")
            nc.scalar.activation(s4, m8[:, 0:K], AF.Exp, bias=negm,
                                 scale=1.0)
            den = small.tile([T, 1], f32, tag="den")
            nc.vector.reduce_sum(den, s4, axis=mybir.AxisListType.X)
            rden = small.tile([T, 1], f32, tag="rden")
            nc.vector.reciprocal(rden, den)
            ew = small.tile([T, K], f32, tag="ew")
            nc.vector.tensor_scalar_mul(ew, s4, rden)

            A = small.tile([T, E], f32, tag="A")
            for k in range(K):
                msk = small.tile([T, E], f32, tag="msk")
                nc.vector.tensor_scalar(msk, g_sb, m8[:, k:k + 1], None,
                                        op0=OP.is_equal)
                wm = small.tile([T, E], f32, tag="wm")
                nc.vector.tensor_scalar_mul(wm, msk, ew[:, k:k + 1])
                if k == 0:
                    nc.vector.tensor_copy(A, wm)
                else:
                    nc.vector.tensor_add(A, A, wm)
            # per-token columns folding routing weight + descales into the
            # lin-side affine op: l3 = A_sb*l2 + A_bb = (BETA/S1)*A*(l2/S1+1)
            A_bb = small.tile([T, K], f32, tag="A_bb")
            nc.vector.tensor_scalar_mul(A_bb, A[:, 0:K], BETA / S1)
            A_sb = small.tile([T, K], f32, tag="A_sb")
            nc.vector.tensor_scalar_mul(A_sb, A[:, 0:K], BETA / (S1 * S1))
            A_hf = small.tile([T, K], f16, tag="A_hf")
            nc.vector.tensor_copy(A_hf, A[:, 0:K])

            # ---- experts: dense masked GLU FFN, fp8 DoubleRow ----
            with (
                tc.tile_pool(name="ps_h", bufs=1, space="PSUM") as ps_h,
                tc.tile_pool(name="ps_tr", bufs=1, space="PSUM") as ps_tr,
            ):
                o1 = ps_o.tile([T, 512], f32, tag="o1")
                o2 = ps_o.tile([T, 128], f32, tag="o2")

                def emit_h(e):
                    w1_t = w1_tiles[e]
                    hp = {}
                    # bias rank-1 first (one stationary load of ones),
                    # then chunk-major DoubleRow pairs sharing stationary
                    for (tag, ofs, n, nb) in H_TILES:
                        pt = ps_h.tile([T, n], f32, tag=tag, bufs=nb)
                        hp[tag] = pt
                        b1s = b1_sb[0:1, 2 * I * e + ofs:2 * I * e + ofs + n]
                        nc.tensor.matmul(pt, ones_hf, b1s,
                                         start=True, stop=False)
                    for d0 in (0, 2):
                        lhs = normed_f8[:, d0:d0 + 2, :]
                        for (tag, ofs, n, _) in H_TILES:
                            nc.tensor.matmul(
                                hp[tag], lhs,
                                w1_t[:, d0:d0 + 2, ofs:ofs + n],
                                perf_mode=DR, start=False, stop=False)
                    lhs = normed_f8[:, 4, :]
                    for (tag, ofs, n, _) in H_TILES:
                        nc.tensor.matmul(hp[tag], lhs,
                                         w1_t[:, 4, ofs:ofs + n],
                                         start=False, stop=True)
                    return hp

                def emit_act(e, hp):
                    # hact = (BETA/S1)*A * gm*sigmoid(BETA*gm/S1) * (l2/S1+1)
                    #      = BETA*A*hact_true; A + all descales fold into
                    # the per-token AP scale/bias of the Identity op.
                    hact_b = hpool.tile([T, 512], f16, tag="hact_b")
                    hact_s = hpool.tile([T, 128], f16, tag="hact_s")
                    for (n, gl, ln, ha) in (
                        (512, hp["hgb"], hp["hlb"], hact_b),
                        (128, hp["hsm"][:, 0:128], hp["hsm"][:, 128:256],
                         hact_s),
                    ):
                        gm = hpool.tile([T, n], f32, tag=f"gm{n}")
                        nc.vector.tensor_scalar(gm, gl, LIMIT * S1, None,
                                                op0=OP.min)
                        sg = hpool.tile([T, n], f32, tag=f"sg{n}")
                        nc.scalar.activation(sg, gm, AF.Sigmoid,
                                             scale=BETA / S1)
                        p_ = hpool.tile([T, n], f32, tag=f"p{n}")
                        nc.vector.tensor_mul(p_, gm, sg)
                        l2 = hpool.tile([T, n], f32, tag=f"l{n}")
                        nc.vector.tensor_scalar(l2, ln, LIMIT * S1,
                                                -LIMIT * S1,
                                                op0=OP.min, op1=OP.max)
                        l3 = hpool.tile([T, n], f32, tag=f"l3{n}")
                        nc.scalar.activation(l3, l2, AF.Identity,
                                             bias=A_bb[:, e:e + 1],
                                             scale=A_sb[:, e:e + 1])
                        nc.vector.tensor_mul(ha, p_, l3)
                    return hact_b, hact_s

                def emit_mm2(e, hact_b, hact_s, last):
                    w2_t = w2_tiles[e]
                    hT = hpool.tile([128, NCH, T], f8, tag="hT")
                    for c in range(NCH):
                        src = (hact_b[:, 128 * c:128 * (c + 1)]
                               if c < 4 else hact_s)
                        pt = ps_tr.tile([128, T], f16, tag="tr")
                        nc.tensor.transpose(pt, src, id_hf)
                        nc.scalar.copy(hT[:, c, :], pt)
                    for c0 in (0, 2):
                        lhs = hT[:, c0:c0 + 2, :]
                        nc.tensor.matmul(o1, lhs,
                                         w2_t[:, c0:c0 + 2, 0:512],
                                         perf_mode=DR,
                                         start=False, stop=False)
                        nc.tensor.matmul(o2, lhs,
                                         w2_t[:, c0:c0 + 2, 512:640],
                                         perf_mode=DR,
                                         start=False, stop=False)
                    nc.tensor.matmul(o1, hT[:, 4, :], w2_t[:, 4, 0:512],
                                     start=False, stop=last)
                    nc.tensor.matmul(o2, hT[:, 4, :], w2_t[:, 4, 512:640],
                                     start=False, stop=last)

                hp0 = emit_h(0)
                # A-weighted b2 bias opens the o1/o2 accumulation groups;
                # emitted after expert 0's h-groups so the router-gated PE
                # work never blocks the DMA-gated first matmuls.
                ps_a = ps_tr.tile([K, T], f16, tag="tr")
                nc.tensor.transpose(ps_a, A_hf, id_hf)
                a4t = small.tile([K, T], f16, tag="a4t")
                nc.scalar.copy(a4t, ps_a)
                nc.tensor.matmul(o1, a4t, b2_t[:, 0:512],
                                 start=True, stop=False)
                nc.tensor.matmul(o2, a4t, b2_t[:, 512:640],
                                 start=True, stop=False)

                hb, hs = emit_act(0, hp0)
                emit_mm2(0, hb, hs, last=False)
                for e in range(1, EPC):
                    hp = emit_h(e)
                    hb, hs = emit_act(e, hp)
                    emit_mm2(e, hb, hs, last=(e == EPC - 1))

                acc = consts.tile([T, D], f32)
                nc.scalar.activation(acc[:, 0:512], o1, AF.Copy,
                                     scale=1.0 / (S2 * BETA))
                nc.scalar.activation(acc[:, 512:640], o2, AF.Copy,
                                     scale=1.0 / (S2 * BETA))

            nc.scalar.dma_start(out=dout.ap(), in_=acc)

    nc.finalize()
    return nc


def _get_nc():
    global _NC
    if _NC is None:
        _ensure_ntff_hook()
        _NC = _build()
    return _NC


# w1/b1 column permutation: [glu 0:512 | lin 0:512 | glu 512:640 | lin ...]
_HPERM = np.r_[0:512, 640:1152, 512:640, 1152:1280]


def _prep_weights(w1, b1, w2, b2):
    """Host-side scale + fp8 cast + tiling. Returns full-E arrays."""
    import ml_dtypes
    f8 = ml_dtypes.float8_e4m3
    w1q = (np.asarray(w1, np.float32)[:, :, _HPERM] * S1).astype(f8)
    b1q = (np.asarray(b1, np.float32)[:, _HPERM] * S1).astype(np.float16)
    w2q = (np.asarray(w2, np.float32) * S2).astype(f8)
    b2q = (np.asarray(b2, np.float32) * (S2 * BETA)).astype(np.float16)
    return w1q, b1q, w2q, b2q


def kernel(**inputs):
    global LAST_EXEC_NS
    nc = _get_nc()
    from concourse.bass_utils import run_bass_kernel_spmd

    x = np.ascontiguousarray(np.asarray(inputs["x"], dtype=np.float32))
    norm_w = np.ascontiguousarray(np.asarray(inputs["norm_w"], np.float32))
    gate_w = np.ascontiguousarray(np.asarray(inputs["gate_w"], np.float32))
    gate_b = np.ascontiguousarray(np.asarray(inputs["gate_b"], np.float32))
    w1q, b1q, w2q, b2q = _prep_weights(inputs["w1"], inputs["b1"],
                                       inputs["w2"], inputs["b2"])

    x2 = np.ascontiguousarray(x[0, :, 0, :])  # (D, T)
    in_maps = []
    for c in range(NCORES):
        lo, hi = EPC * c, EPC * (c + 1)
        perm = np.r_[lo:hi, 0:lo, hi:E]
        # (EPC, D, F) -> (128, EPC, NCH, F): partition p holds row c*128+p
        w1c = np.ascontiguousarray(
            w1q[lo:hi].reshape(EPC, NCH, 128, 2 * I).transpose(2, 0, 1, 3))
        w2c = np.ascontiguousarray(
            w2q[lo:hi].reshape(EPC, NCH, 128, D).transpose(2, 0, 1, 3))
        in_maps.append({
            "x": x2,
            "norm_w": norm_w,
            "gate_w": np.ascontiguousarray(gate_w[perm]),
            "gate_b": np.ascontiguousarray(gate_b[perm]),
            "w1": w1c,
            "b1": np.ascontiguousarray(b1q[lo:hi]),
            "w2": w2c,
            "b2": np.ascontiguousarray(b2q[lo:hi]),
        })

    res = run_bass_kernel_spmd(nc, in_maps, core_ids=list(range(NCORES)),
                               trace=TRACE, tmpdir=PROF_DIR)
    LAST_EXEC_NS = res.exec_time_ns
    total = np.sum([r["out"] for r in res.results], axis=0)  # (T, D)
    return (x + total.T[None, :, None, :]).astype(np.float32)


# revision 9
# speedup vs baseline: 1.0364x; 1.0364x over previous
"""MoE block (RMSNorm + top-4 router + 32-expert GLU FFN) on 8 TRN2 NeuronCores.

Expert-parallel: core c owns experts [4c, 4c+4). Each core redundantly
computes the RMSNorm + router over all 32 experts (fp16 gate matmul, f32
top-k/softmax), then runs a dense masked GLU FFN over all 64 tokens for its
own 4 experts with fp8(e4m3) weights and DoubleRow (2x-rate) matmuls.

Memory-bound problem: the weight stream (w1/w2) dominates. Weights are
host-cast to fp8 e4m3 with a power-of-2 scale (S=256 centers the 0.02-std
values in the e4m3 normal range) and host-pre-tiled to (128, e, c, i) so
each per-expert DMA is one fully-contiguous-per-partition transfer. w1
loads ride the sync HWDGE ring, w2 loads the scalar HWDGE ring, so trigger
issue is parallel and expert e's w2 lands before its mm2 needs it.

PE-side structure (the engine that bounds compute here):
- ~4us of dummy fp16 matmuls at kernel start keep the PE busy through the
  DMA-wait window so the HAM clock-gate opens (1.2 -> 2.4 GHz) before the
  expert phase. The router runs after them, warm.
- gate_w.T is host-pretransposed (kills 5 cold fp32 PE transposes), the
  RMSNorm square-sum matmul runs in fp16 (kills fp32 double-pass loads).
- ACT tables load in first-use order (Sqrt before RMSNorm, Exp after,
  Sigmoid/Identity during the expert ramp) so no table load sits in front
  of a critical-path op; all copies use Identity so no Copy table at all.
- Expert emission is software-pipelined: expert e+1's rank-1 b1 matmuls
  issue between expert e's mm1 and mm2 so the PE has ready work while the
  DVE/ACT activation stage of expert e drains.

Scale bookkeeping: h' = S1*(t@w1+b1) in psum. glu: p = gm*sigmoid(gm*
BETA/S1), gm = min(h',7*S1). lin: l3 = (BETA/S1)*A*(l2/S1+1) via one ACT
Identity op with per-token AP scale/bias (A = routing weight; folds the
expert mix into hact). hact = p*l3 = BETA*A*hact_true, cast fp8; mm2 with
w2*S2 accumulates S2*BETA*sum_e A_e*(hact@w2) for all 4 experts plus the
A-weighted b2 bias (host-scaled by S2*BETA) into one psum group; the final
copy descales by 1/(S2*BETA). Host sums the 8 per-core partials and adds
the residual.
"""

import sys
import types

sys.path.insert(0, "/opt/trn_rl_repo")

import numpy as np

D = 640
I = 640
E = 32
T = 64
K = 4
EPS = 1e-5
LIMIT = 7.0
BETA = 1.702
NCORES = 8
EPC = E // NCORES          # experts per core
NCH = D // 128             # 5 contraction chunks of 128
S1 = 256.0                 # host scale on w1/b1
S2 = 256.0                 # host scale on w2
NWARM = 10                 # dummy 512-wide fp16 matmuls to open the HAM gate

TRACE = False
PROF_DIR = None
LAST_EXEC_NS = None

_NC = None


def _ensure_ntff_hook():
    """boot() skips NTFF hook registration (image antenv lacks axon_hooks);
    provide the module so bass_utils can profile when TRACE=True."""
    if "antenv.axon_hooks" in sys.modules:
        return
    try:
        from trn_agent_boot.trn_boot import _ntff_profile_via_ctypes
        hook = _ntff_profile_via_ctypes("/opt/axon/libaxon_pjrt.so")
    except Exception:
        hook = None
    m = types.ModuleType("antenv.axon_hooks")
    m.get_axon_ntff_profile_hook = lambda: hook
    m.set_axon_ntff_profile_hook = lambda h: None
    sys.modules["antenv.axon_hooks"] = m


# h psum tiles after the host column permutation: contiguous slices.
# (tag, col offset, width, bufs). hsm = [glu 512:640 | lin 512:640].
H_TILES = [("hgb", 0, 512, 2), ("hlb", 512, 512, 2), ("hsm", 1024, 256, 1)]


def _build():
    import concourse.bass as bass
    import concourse.bacc as bacc
    import concourse.tile as tile
    from concourse import mybir
    from concourse.masks import make_identity

    f32 = mybir.dt.float32
    f16 = mybir.dt.float16
    f8 = mybir.dt.float8e4
    AF = mybir.ActivationFunctionType
    OP = mybir.AluOpType
    DR = mybir.MatmulPerfMode.DoubleRow

    nc = bacc.Bacc("TRN2", target_bir_lowering=False, debug=False,
                   num_devices=NCORES)
    dx = nc.dram_tensor("x", (128, NCH, T), f32, kind="ExternalInput")
    dnw = nc.dram_tensor("norm_w", (128, NCH), f32, kind="ExternalInput")
    dgwt = nc.dram_tensor("gate_wt", (128, NCH, E), f32,
                          kind="ExternalInput")
    dgb = nc.dram_tensor("gate_b", (E,), f32, kind="ExternalInput")
    dw1 = nc.dram_tensor("w1", (128, EPC, NCH, 2 * I), f8,
                         kind="ExternalInput")
    db1 = nc.dram_tensor("b1", (EPC, 2 * I), f16, kind="ExternalInput")
    dw2 = nc.dram_tensor("w2", (128, EPC, NCH, D), f8, kind="ExternalInput")
    db2 = nc.dram_tensor("b2", (EPC, D), f16, kind="ExternalInput")
    dout = nc.dram_tensor("out", (T, D), f32, kind="ExternalOutput")

    with tile.TileContext(nc) as tc:
        with (
            tc.tile_pool(name="consts", bufs=1) as consts,
            tc.tile_pool(name="small", bufs=2) as small,
            tc.tile_pool(name="hpool", bufs=2) as hpool,
            tc.tile_pool(name="ps_o", bufs=1, space="PSUM") as ps_o,
        ):
            # ---- sync HWDGE ring: x then the w1 stream ----
            x_t = consts.tile([128, NCH, T], f32)
            nc.sync.dma_start(out=x_t, in_=dx.ap())
            w1_tiles = [consts.tile([128, NCH, 2 * I], f8, tag=f"w1_{e}",
                                    name=f"w1t{e}") for e in range(EPC)]
            w2_tiles = [consts.tile([128, NCH, D], f8, tag=f"w2_{e}",
                                    name=f"w2t{e}") for e in range(EPC)]
            for e in range(EPC):
                nc.sync.dma_start(out=w1_tiles[e], in_=dw1.ap()[:, e])
            # ---- scalar HWDGE ring: gate_w.T then w2_0 (w2_1..3 triggers
            # are emitted after the router so no ACT-queue op blocks the
            # critical-path table loads / activations) ----
            gwt_t = consts.tile([128, NCH, E], f32)
            nc.scalar.dma_start(out=gwt_t, in_=dgwt.ap())
            nc.scalar.dma_start(out=w2_tiles[0], in_=dw2.ap()[:, 0])

            b1_sb = consts.tile([1, EPC * 2 * I], f16)
            nc.gpsimd.dma_start(out=b1_sb,
                                in_=db1.ap().rearrange("e i -> (e i)")[None, :])
            nw_t = consts.tile([128, NCH], f32)
            nc.gpsimd.dma_start(out=nw_t, in_=dnw.ap())
            gb_b = consts.tile([T, E], f32)
            gb_base = dgb.ap()
            nc.gpsimd.dma_start(
                out=gb_b,
                in_=bass.AP(tensor=gb_base.tensor, offset=0,
                            ap=[[0, T], [1, E]]))
            b2_t = consts.tile([EPC, D], f16)
            nc.gpsimd.dma_start(out=b2_t, in_=db2.ap())

            ones_hf128 = consts.tile([128, 128], f16)
            nc.vector.memset(ones_hf128, 1.0)
            ones_hf = consts.tile([1, T], f16)
            nc.vector.memset(ones_hf, 1.0)
            eps_t = consts.tile([128, 1], f32)
            nc.vector.memset(eps_t, EPS)
            warm_src = consts.tile([128, 512], f16)
            nc.vector.memset(warm_src, 0.0)
            id_hf = consts.tile([T, T], f16)
            make_identity(nc, id_hf)

            with tc.tile_pool(name="ps_misc", bufs=2, space="PSUM") as ps_misc:
                # ---- PE warm-up: keep the PE busy through the DMA-wait
                # window so HAM unthrottles before real work ----
                ps_w = ps_misc.tile([128, 512], f32, tag="warm", bufs=1)
                for _ in range(NWARM):
                    nc.tensor.matmul(ps_w, ones_hf128, warm_src,
                                     start=True, stop=True)

                # Sqrt table first -- the only ACT op before RMSNorm's sq
                dmy0 = consts.tile([1, 1], f32, tag="dmy0")
                nc.scalar.activation(dmy0, eps_t[0:1, :], AF.Sqrt)

                # ---- RMSNorm (x is (D, T); D on partitions); fp16 ss ----
                xx = small.tile([128, NCH, T], f16, tag="xx")
                nc.vector.tensor_mul(xx, x_t, x_t)
                ps_ss = ps_misc.tile([128, T], f32, tag="misc")
                for c in range(NCH):
                    nc.tensor.matmul(ps_ss, ones_hf128, xx[:, c, :],
                                     start=(c == 0), stop=(c == NCH - 1))
                sq = small.tile([128, T], f32, tag="sq")
                nc.scalar.activation(sq, ps_ss, AF.Sqrt, bias=eps_t,
                                     scale=1.0 / D)
                # next-needed ACT table while DVE normalizes
                dmy1 = consts.tile([1, 1], f32, tag="dmy1")
                nc.scalar.activation(dmy1, eps_t[0:1, :], AF.Exp)
                rstd = small.tile([128, T], f32, tag="rstd")
                nc.vector.reciprocal(rstd, sq)
                normed_hf = consts.tile([128, NCH, T], f16)
                normed_f8 = consts.tile([128, NCH, T], f8)
                for c in range(NCH):
                    xn = small.tile([128, T], f32, tag="xn")
                    nc.vector.tensor_scalar_mul(xn, x_t[:, c, :],
                                                nw_t[:, c:c + 1])
                    nc.vector.tensor_mul(normed_hf[:, c, :], xn, rstd)
                    nc.vector.tensor_mul(normed_f8[:, c, :], xn, rstd)

                # ---- router: gate, top-4, softmax, routing matrix A ----
                gwT_hf = consts.tile([128, NCH, E], f16)
                nc.vector.tensor_copy(gwT_hf, gwt_t)
                ps_g = ps_misc.tile([T, E], f32, tag="misc")
                for c in range(NCH):
                    nc.tensor.matmul(ps_g, normed_hf[:, c, :], gwT_hf[:, c, :],
                                     start=(c == 0), stop=(c == NCH - 1))
                g_sb = small.tile([T, E], f32, tag="g")
                nc.vector.tensor_add(g_sb, ps_g, gb_b)

            m8 = small.tile([T, 8], f32, tag="m8")
            nc.vector.max(m8, g_sb)
            negm = small.tile([T, 1], f32, tag="negm")
            nc.vector.tensor_scalar_mul(negm, m8[:, 0:1], -1.0)
            s4 = small.tile([T, K], f32, tag="s4")
            nc.scalar.activation(s4, m8[:, 0:K], AF.Exp, bias=negm,
                                 scale=1.0)
            den = small.tile([T, 1], f32, tag="den")
            nc.vector.reduce_sum(den, s4, axis=mybir.AxisListType.X)
            rden = small.tile([T, 1], f32, tag="rden")
            nc.vector.reciprocal(rden, den)
            ew = small.tile([T, K], f32, tag="ew")
            nc.vector.tensor_scalar_mul(ew, s4, rden)

            # remaining w2 triggers + remaining ACT tables, in first-use
            # order, now that the router's scalar ops are queued
            for e in range(1, EPC):
                nc.scalar.dma_start(out=w2_tiles[e], in_=dw2.ap()[:, e])
            dmy2 = consts.tile([1, 1], f32, tag="dmy2")
            nc.scalar.activation(dmy2, eps_t[0:1, :], AF.Sigmoid)
            dmy3 = consts.tile([1, 1], f32, tag="dmy3")
            nc.scalar.activation(dmy3, eps_t[0:1, :], AF.Identity)

            A = small.tile([T, E], f32, tag="A")
            for k in range(K):
                msk = small.tile([T, E], f32, tag="msk")
                nc.vector.tensor_scalar(msk, g_sb, m8[:, k:k + 1], None,
                                        op0=OP.is_equal)
                wm = small.tile([T, E], f32, tag="wm")
                nc.vector.tensor_scalar_mul(wm, msk, ew[:, k:k + 1])
                if k == 0:
                    nc.vector.tensor_copy(A, wm)
                else:
                    nc.vector.tensor_add(A, A, wm)
            # per-token columns folding routing weight + descales into the
            # lin-side affine op: l3 = A_sb*l2 + A_bb = (BETA/S1)*A*(l2/S1+1)
            A_bb = small.tile([T, K], f32, tag="A_bb")
            nc.vector.tensor_scalar_mul(A_bb, A[:, 0:K], BETA / S1)
            A_sb = small.tile([T, K], f32, tag="A_sb")
            nc.vector.tensor_scalar_mul(A_sb, A[:, 0:K], BETA / (S1 * S1))
            A_hf = small.tile([T, K], f16, tag="A_hf")
            nc.vector.tensor_copy(A_hf, A[:, 0:K])

            # ---- experts: dense masked GLU FFN, fp8 DoubleRow, software
            # pipelined so the PE always has ready work ----
            with (
                tc.tile_pool(name="ps_h", bufs=1, space="PSUM") as ps_h,
                tc.tile_pool(name="ps_tr", bufs=1, space="PSUM") as ps_tr,
            ):
                o1 = ps_o.tile([T, 512], f32, tag="o1")
                o2 = ps_o.tile([T, 128], f32, tag="o2")

                def emit_h_bias(e):
                    # rank-1 b1 rows open each h psum group; no data deps
                    # beyond b1, so these fill PE gaps while DVE/ACT drain
                    # the previous expert
                    hp = {}
                    for (tag, ofs, n, nb) in H_TILES:
                        pt = ps_h.tile([T, n], f32, tag=tag, bufs=nb,
                                       name=f"h_{tag}_{e}")
                        hp[tag] = pt
                        b1s = b1_sb[0:1, 2 * I * e + ofs:2 * I * e + ofs + n]
                        nc.tensor.matmul(pt, ones_hf, b1s,
                                         start=True, stop=False)
                    return hp

                def emit_h_mm(e, hp):
                    w1_t = w1_tiles[e]
                    for d0 in (0, 2):
                        lhs = normed_f8[:, d0:d0 + 2, :]
                        for (tag, ofs, n, _) in H_TILES:
                            nc.tensor.matmul(
                                hp[tag], lhs,
                                w1_t[:, d0:d0 + 2, ofs:ofs + n],
                                perf_mode=DR, start=False, stop=False)
                    lhs = normed_f8[:, 4, :]
                    for (tag, ofs, n, _) in H_TILES:
                        nc.tensor.matmul(hp[tag], lhs,
                                         w1_t[:, 4, ofs:ofs + n],
                                         start=False, stop=True)

                def emit_act(e, hp):
                    # hact = (BETA/S1)*A * gm*sigmoid(BETA*gm/S1) * (l2/S1+1)
                    #      = BETA*A*hact_true
                    hact_b = hpool.tile([T, 512], f16, tag="hact_b",
                                        name=f"hab{e}")
                    hact_s = hpool.tile([T, 128], f16, tag="hact_s",
                                        name=f"has{e}")
                    for (n, gl, ln, ha) in (
                        (512, hp["hgb"], hp["hlb"], hact_b),
                        (128, hp["hsm"][:, 0:128], hp["hsm"][:, 128:256],
                         hact_s),
                    ):
                        gm = hpool.tile([T, n], f32, tag=f"gm{n}",
                                        name=f"gm{n}_{e}")
                        nc.vector.tensor_scalar(gm, gl, LIMIT * S1, None,
                                                op0=OP.min)
                        sg = hpool.tile([T, n], f32, tag=f"sg{n}",
                                        name=f"sg{n}_{e}")
                        nc.scalar.activation(sg, gm, AF.Sigmoid,
                                             scale=BETA / S1)
                        p_ = hpool.tile([T, n], f32, tag=f"p{n}",
                                        name=f"p{n}_{e}")
                        nc.vector.tensor_mul(p_, gm, sg)
                        l2 = hpool.tile([T, n], f32, tag=f"l{n}",
                                        name=f"l{n}_{e}")
                        nc.vector.tensor_scalar(l2, ln, LIMIT * S1,
                                                -LIMIT * S1,
                                                op0=OP.min, op1=OP.max)
                        l3 = hpool.tile([T, n], f32, tag=f"l3{n}",
                                        name=f"l3{n}_{e}")
                        nc.scalar.activation(l3, l2, AF.Identity,
                                             bias=A_bb[:, e:e + 1],
                                             scale=A_sb[:, e:e + 1])
                        nc.vector.tensor_mul(ha, p_, l3)
                    return hact_b, hact_s

                def emit_mm2(e, hact_b, hact_s, last):
                    w2_t = w2_tiles[e]
                    hT = hpool.tile([128, NCH, T], f8, tag="hT",
                                    name=f"hT{e}")
                    for c in range(NCH):
                        src = (hact_b[:, 128 * c:128 * (c + 1)]
                               if c < 4 else hact_s)
                        pt = ps_tr.tile([128, T], f16, tag="tr",
                                        name=f"tr{e}_{c}")
                        nc.tensor.transpose(pt, src, id_hf)
                        nc.scalar.activation(hT[:, c, :], pt, AF.Identity)
                    for c0 in (0, 2):
                        lhs = hT[:, c0:c0 + 2, :]
                        nc.tensor.matmul(o1, lhs,
                                         w2_t[:, c0:c0 + 2, 0:512],
                                         perf_mode=DR,
                                         start=False, stop=False)
                        nc.tensor.matmul(o2, lhs,
                                         w2_t[:, c0:c0 + 2, 512:640],
                                         perf_mode=DR,
                                         start=False, stop=False)
                    nc.tensor.matmul(o1, hT[:, 4, :], w2_t[:, 4, 0:512],
                                     start=False, stop=last)
                    nc.tensor.matmul(o2, hT[:, 4, :], w2_t[:, 4, 512:640],
                                     start=False, stop=last)

                hp = emit_h_bias(0)
                emit_h_mm(0, hp)
                # A-weighted b2 bias opens the o1/o2 accumulation groups
                ps_a = ps_tr.tile([K, T], f16, tag="tr", name="tra")
                nc.tensor.transpose(ps_a, A_hf, id_hf)
                a4t = small.tile([K, T], f16, tag="a4t")
                nc.scalar.activation(a4t, ps_a, AF.Identity)
                nc.tensor.matmul(o1, a4t, b2_t[:, 0:512],
                                 start=True, stop=False)
                nc.tensor.matmul(o2, a4t, b2_t[:, 512:640],
                                 start=True, stop=False)

                ha = emit_act(0, hp)
                for e in range(1, EPC):
                    hp_next = emit_h_bias(e)
                    emit_mm2(e - 1, *ha, last=False)
                    emit_h_mm(e, hp_next)
                    ha = emit_act(e, hp_next)
                emit_mm2(EPC - 1, *ha, last=True)

                acc = consts.tile([T, D], f32)
                nc.scalar.activation(acc[:, 0:512], o1, AF.Identity,
                                     scale=1.0 / (S2 * BETA))
                nc.scalar.activation(acc[:, 512:640], o2, AF.Identity,
                                     scale=1.0 / (S2 * BETA))

            nc.scalar.dma_start(out=dout.ap(), in_=acc)

    nc.finalize()
    return nc


def _get_nc():
    global _NC
    if _NC is None:
        _ensure_ntff_hook()
        _NC = _build()
    return _NC


# w1/b1 column permutation: [glu 0:512 | lin 0:512 | glu 512:640 | lin ...]
_HPERM = np.r_[0:512, 640:1152, 512:640, 1152:1280]


def _prep_weights(w1, b1, w2, b2):
    """Host-side scale + fp8 cast + tiling. Returns full-E arrays."""
    import ml_dtypes
    f8 = ml_dtypes.float8_e4m3
    w1q = (np.asarray(w1, np.float32)[:, :, _HPERM] * S1).astype(f8)
    b1q = (np.asarray(b1, np.float32)[:, _HPERM] * S1).astype(np.float16)
    w2q = (np.asarray(w2, np.float32) * S2).astype(f8)
    b2q = (np.asarray(b2, np.float32) * (S2 * BETA)).astype(np.float16)
    return w1q, b1q, w2q, b2q


def _ptile(a, nch):
    """(rows, F) -> (128, nch, F): partition p holds row c*128+p."""
    rows, F = a.shape
    return np.ascontiguousarray(
        a.reshape(nch, 128, F).transpose(1, 0, 2))


def kernel(**inputs):
    global LAST_EXEC_NS
    nc = _get_nc()
    from concourse.bass_utils import run_bass_kernel_spmd

    x = np.ascontiguousarray(np.asarray(inputs["x"], dtype=np.float32))
    norm_w = np.asarray(inputs["norm_w"], np.float32)
    gate_w = np.asarray(inputs["gate_w"], np.float32)
    gate_b = np.ascontiguousarray(np.asarray(inputs["gate_b"], np.float32))
    w1q, b1q, w2q, b2q = _prep_weights(inputs["w1"], inputs["b1"],
                                       inputs["w2"], inputs["b2"])

    x2 = np.ascontiguousarray(x[0, :, 0, :])  # (D, T)
    xt = _ptile(x2, NCH)
    nwt = np.ascontiguousarray(norm_w.reshape(NCH, 128).T)
    in_maps = []
    for c in range(NCORES):
        lo, hi = EPC * c, EPC * (c + 1)
        perm = np.r_[lo:hi, 0:lo, hi:E]
        # (EPC, D, F) -> (128, EPC, NCH, F): partition p holds row c*128+p
        w1c = np.ascontiguousarray(
            w1q[lo:hi].reshape(EPC, NCH, 128, 2 * I).transpose(2, 0, 1, 3))
        w2c = np.ascontiguousarray(
            w2q[lo:hi].reshape(EPC, NCH, 128, D).transpose(2, 0, 1, 3))
        in_maps.append({
            "x": xt,
            "norm_w": nwt,
            "gate_wt": _ptile(np.ascontiguousarray(gate_w[perm].T), NCH),
            "gate_b": np.ascontiguousarray(gate_b[perm]),
            "w1": w1c,
            "b1": np.ascontiguousarray(b1q[lo:hi]),
            "w2": w2c,
            "b2": np.ascontiguousarray(b2q[lo:hi]),
        })

    res = run_bass_kernel_spmd(nc, in_maps, core_ids=list(range(NCORES)),
                               trace=TRACE, tmpdir=PROF_DIR)
    LAST_EXEC_NS = res.exec_time_ns
    total = np.sum([r["out"] for r in res.results], axis=0)  # (T, D)
    return (x + total.T[None, :, None, :]).astype(np.float32)


# revision 16
# speedup vs baseline: 1.1857x; 1.1441x over previous
"""MoE block (RMSNorm + top-4 router + 32-expert GLU FFN) on 8 TRN2 NeuronCores.

Expert-parallel: core c owns experts [4c, 4c+4). Each core redundantly
computes the RMSNorm + router over all 32 experts (fp16 gate matmul, f32
top-k/softmax), then runs a dense masked GLU FFN over all 64 tokens for its
own 4 experts with fp8(e4m3) weights and DoubleRow (2x-rate) matmuls.

Memory-bound problem: the weight stream (w1/w2) dominates. Weights are
host-cast to fp8 e4m3 with a power-of-2 scale (S=256 centers the 0.02-std
values in the e4m3 normal range) and host-pre-tiled to (128, e, c, i) so
each per-expert DMA is one fully-contiguous-per-partition transfer. w1
loads ride the sync HWDGE ring, w2 loads the scalar HWDGE ring, so trigger
issue is parallel and expert e's w2 lands before its mm2 needs it.

PE-side structure (the engine that bounds compute here):
- ~4us of dummy fp16 matmuls at kernel start keep the PE busy through the
  DMA-wait window so the HAM clock-gate opens (1.2 -> 2.4 GHz) before the
  expert phase. The router runs after them, warm.
- gate_w.T is host-pretransposed (kills 5 cold fp32 PE transposes), the
  RMSNorm square-sum matmul runs in fp16 (kills fp32 double-pass loads).
- ACT tables load in first-use order (Sqrt before RMSNorm, Exp after,
  Sigmoid/Identity during the expert ramp) so no table load sits in front
  of a critical-path op; all copies use Identity so no Copy table at all.
- Expert emission is software-pipelined: expert e+1's rank-1 b1 matmuls
  issue between expert e's mm1 and mm2 so the PE has ready work while the
  DVE/ACT activation stage of expert e drains.

Scale bookkeeping: h' = S1*(t@w1+b1) in psum. glu: p = gm*sigmoid(gm*
BETA/S1), gm = min(h',7*S1). lin: l3 = (BETA/S1)*A*(l2/S1+1) via one ACT
Identity op with per-token AP scale/bias (A = routing weight; folds the
expert mix into hact). hact = p*l3 = BETA*A*hact_true, cast fp8; mm2 with
w2*S2 accumulates S2*BETA*sum_e A_e*(hact@w2) for all 4 experts plus the
A-weighted b2 bias (host-scaled by S2*BETA) into one psum group; the final
copy descales by 1/(S2*BETA). Host sums the 8 per-core partials and adds
the residual.
"""

import sys
import types

sys.path.insert(0, "/opt/trn_rl_repo")

import numpy as np

D = 640
I = 640
E = 32
T = 64
K = 4
EPS = 1e-5
LIMIT = 7.0
BETA = 1.702
NCORES = 8
EPC = E // NCORES          # experts per core
NCH = D // 128             # 5 contraction chunks of 128
S1 = 256.0                 # host scale on w1/b1
S2 = 256.0                 # host scale on w2
NWARM = 10                 # dummy 512-wide fp16 matmuls to open the HAM gate

TRACE = False
PROF_DIR = None
LAST_EXEC_NS = None

_NC = None


def _ensure_ntff_hook():
    """boot() skips NTFF hook registration (image antenv lacks axon_hooks);
    provide the module so bass_utils can profile when TRACE=True."""
    if "antenv.axon_hooks" in sys.modules:
        return
    try:
        from trn_agent_boot.trn_boot import _ntff_profile_via_ctypes
        hook = _ntff_profile_via_ctypes("/opt/axon/libaxon_pjrt.so")
    except Exception:
        hook = None
    m = types.ModuleType("antenv.axon_hooks")
    m.get_axon_ntff_profile_hook = lambda: hook
    m.set_axon_ntff_profile_hook = lambda h: None
    sys.modules["antenv.axon_hooks"] = m


# h psum tiles after the host column permutation: contiguous slices.
# (tag, col offset, width, bufs). hsm = [glu 512:640 | lin 512:640].
H_TILES = [("hgb", 0, 512, 2), ("hlb", 512, 512, 2), ("hsm", 1024, 256, 1)]


def _build():
    import concourse.bass as bass
    import concourse.bacc as bacc
    import concourse.tile as tile
    from concourse import mybir
    from concourse.masks import make_identity

    f32 = mybir.dt.float32
    f16 = mybir.dt.float16
    f8 = mybir.dt.float8e4
    AF = mybir.ActivationFunctionType
    OP = mybir.AluOpType
    DR = mybir.MatmulPerfMode.DoubleRow

    nc = bacc.Bacc("TRN2", target_bir_lowering=False, debug=False,
                   num_devices=NCORES)
    dx = nc.dram_tensor("x", (128, NCH, T), f32, kind="ExternalInput")
    dnw = nc.dram_tensor("norm_w", (128, NCH), f32, kind="ExternalInput")
    dgwt = nc.dram_tensor("gate_wt", (128, NCH, E), f16,
                          kind="ExternalInput")
    dgb = nc.dram_tensor("gate_b", (E,), f32, kind="ExternalInput")
    dw1 = nc.dram_tensor("w1", (128, EPC, NCH, 2 * I), f8,
                         kind="ExternalInput")
    db1 = nc.dram_tensor("b1", (EPC, 2 * I), f16, kind="ExternalInput")
    dw2 = nc.dram_tensor("w2", (128, EPC, NCH, D), f8, kind="ExternalInput")
    db2 = nc.dram_tensor("b2", (EPC, D), f16, kind="ExternalInput")
    dout = nc.dram_tensor("out", (T, D), f32, kind="ExternalOutput")

    with tile.TileContext(nc) as tc:
        with (
            tc.tile_pool(name="consts", bufs=1) as consts,
            tc.tile_pool(name="small", bufs=2) as small,
            tc.tile_pool(name="hpool", bufs=2) as hpool,
            tc.tile_pool(name="ps_o", bufs=1, space="PSUM") as ps_o,
        ):
            # ---- sync HWDGE ring: x then the w1 stream ----
            x_t = consts.tile([128, NCH, T], f32)
            nc.sync.dma_start(out=x_t, in_=dx.ap())
            w1_tiles = [consts.tile([128, NCH, 2 * I], f8, tag=f"w1_{e}",
                                    name=f"w1t{e}") for e in range(EPC)]
            w2_tiles = [consts.tile([128, NCH, D], f8, tag=f"w2_{e}",
                                    name=f"w2t{e}") for e in range(EPC)]
            for e in range(EPC):
                nc.sync.dma_start(out=w1_tiles[e], in_=dw1.ap()[:, e])

            b1_sb = consts.tile([1, EPC * 2 * I], f16)
            nc.gpsimd.dma_start(out=b1_sb,
                                in_=db1.ap().rearrange("e i -> (e i)")[None, :])
            nw_t = consts.tile([128, NCH], f32)
            nc.gpsimd.dma_start(out=nw_t, in_=dnw.ap())
            gb_b = consts.tile([T, E], f32)
            gb_base = dgb.ap()
            nc.gpsimd.dma_start(
                out=gb_b,
                in_=bass.AP(tensor=gb_base.tensor, offset=0,
                            ap=[[0, T], [1, E]]))
            b2_t = consts.tile([EPC, D], f16)
            nc.gpsimd.dma_start(out=b2_t, in_=db2.ap())

            ones_hf128 = consts.tile([128, 128], f16)
            nc.vector.memset(ones_hf128, 1.0)
            ones_hf = consts.tile([1, T], f16)
            nc.vector.memset(ones_hf, 1.0)
            eps_t = consts.tile([128, 1], f32)
            nc.vector.memset(eps_t, EPS)
            warm_src = consts.tile([128, 512], f16)
            nc.vector.memset(warm_src, 0.0)
            id_hf = consts.tile([T, T], f16)
            make_identity(nc, id_hf)

            with tc.tile_pool(name="ps_misc", bufs=2, space="PSUM") as ps_misc:
                # Sqrt table first -- the only ACT op before RMSNorm's sq;
                # then the scalar-ring DMA triggers (gate_wt, w2_0)
                dmy0 = consts.tile([1, 1], f32, tag="dmy0")
                nc.scalar.activation(dmy0, eps_t[0:1, :], AF.Sqrt)
                gwt_t = consts.tile([128, NCH, E], f16)
                nc.scalar.dma_start(out=gwt_t, in_=dgwt.ap())
                nc.scalar.dma_start(out=w2_tiles[0], in_=dw2.ap()[:, 0])

                # ---- PE warm-up: keep the PE continuously busy through
                # the DMA-wait window so the HAM clock-gate opens before
                # the expert phase; RMSNorm's square-sum slots between ----
                ps_w = ps_misc.tile([128, 512], f32, tag="warm", bufs=1)
                for _ in range(6):
                    nc.tensor.matmul(ps_w, ones_hf128, warm_src,
                                     start=True, stop=True)

                # ---- RMSNorm (x is (D, T); D on partitions); fp16 ss ----
                xx = small.tile([128, NCH, T], f16, tag="xx")
                nc.vector.tensor_mul(xx, x_t, x_t)
                ps_ss = ps_misc.tile([128, T], f32, tag="misc")
                for c in range(NCH):
                    nc.tensor.matmul(ps_ss, ones_hf128, xx[:, c, :],
                                     start=(c == 0), stop=(c == NCH - 1))
                for _ in range(NWARM - 6):
                    nc.tensor.matmul(ps_w, ones_hf128, warm_src,
                                     start=True, stop=True)
                sq = small.tile([128, T], f32, tag="sq")
                nc.scalar.activation(sq, ps_ss, AF.Sqrt, bias=eps_t,
                                     scale=1.0 / D)
                # next-needed ACT tables while DVE normalizes
                dmy1 = consts.tile([1, 1], f32, tag="dmy1")
                nc.scalar.activation(dmy1, eps_t[0:1, :], AF.Exp)
                dmy2 = consts.tile([1, 1], f32, tag="dmy2")
                nc.scalar.activation(dmy2, eps_t[0:1, :], AF.Sigmoid)
                rstd = small.tile([128, T], f32, tag="rstd")
                nc.vector.reciprocal(rstd, sq)
                normed_hf = consts.tile([128, NCH, T], f16)
                normed_f8 = consts.tile([128, NCH, T], f8)
                for c in range(NCH):
                    xn = small.tile([128, T], f32, tag="xn")
                    nc.vector.tensor_scalar_mul(xn, x_t[:, c, :],
                                                nw_t[:, c:c + 1])
                    nc.vector.tensor_mul(normed_hf[:, c, :], xn, rstd)
                    nc.vector.tensor_mul(normed_f8[:, c, :], xn, rstd)

                # ---- router: gate, top-4, softmax, routing matrix A ----
                ps_g = ps_misc.tile([T, E], f32, tag="misc")
                for c in range(NCH):
                    nc.tensor.matmul(ps_g, normed_hf[:, c, :], gwt_t[:, c, :],
                                     start=(c == 0), stop=(c == NCH - 1))
                g_sb = small.tile([T, E], f32, tag="g")
                nc.vector.tensor_add(g_sb, ps_g, gb_b)

            m8 = small.tile([T, 8], f32, tag="m8")
            nc.vector.max(m8, g_sb)
            negm = small.tile([T, 1], f32, tag="negm")
            nc.vector.tensor_scalar_mul(negm, m8[:, 0:1], -1.0)
            s4 = small.tile([T, K], f32, tag="s4")
            nc.scalar.activation(s4, m8[:, 0:K], AF.Exp, bias=negm,
                                 scale=1.0)
            den = small.tile([T, 1], f32, tag="den")
            nc.vector.reduce_sum(den, s4, axis=mybir.AxisListType.X)
            rden = small.tile([T, 1], f32, tag="rden")
            nc.vector.reciprocal(rden, den)
            ew = small.tile([T, K], f32, tag="ew")
            nc.vector.tensor_scalar_mul(ew, s4, rden)

            # remaining ACT table + w2 triggers, in first-use order
            dmy3 = consts.tile([1, 1], f32, tag="dmy3")
            nc.scalar.activation(dmy3, eps_t[0:1, :], AF.Identity)
            for e in range(1, EPC):
                nc.scalar.dma_start(out=w2_tiles[e], in_=dw2.ap()[:, e])

            A = small.tile([T, E], f32, tag="A")
            for k in range(K):
                msk = small.tile([T, E], f32, tag="msk")
                nc.vector.tensor_scalar(msk, g_sb, m8[:, k:k + 1], None,
                                        op0=OP.is_equal)
                wm = small.tile([T, E], f32, tag="wm")
                nc.vector.tensor_scalar_mul(wm, msk, ew[:, k:k + 1])
                if k == 0:
                    nc.vector.tensor_copy(A, wm)
                else:
                    nc.vector.tensor_add(A, A, wm)
            # per-token columns folding routing weight + descales into the
            # lin-side affine op: l3 = A_sb*l2 + A_bb = (BETA/S1)*A*(l2/S1+1)
            A_bb = small.tile([T, K], f32, tag="A_bb")
            nc.vector.tensor_scalar_mul(A_bb, A[:, 0:K], BETA / S1)
            A_sb = small.tile([T, K], f32, tag="A_sb")
            nc.vector.tensor_scalar_mul(A_sb, A[:, 0:K], BETA / (S1 * S1))
            A_hf = small.tile([T, K], f16, tag="A_hf")
            nc.vector.tensor_copy(A_hf, A[:, 0:K])

            # ---- experts: dense masked GLU FFN, fp8 DoubleRow, software
            # pipelined so the PE always has ready work ----
            with (
                tc.tile_pool(name="ps_h", bufs=1, space="PSUM") as ps_h,
                tc.tile_pool(name="ps_tr", bufs=1, space="PSUM") as ps_tr,
            ):
                o1 = ps_o.tile([T, 512], f32, tag="o1")
                o2 = ps_o.tile([T, 128], f32, tag="o2")

                def emit_h_bias(e):
                    # rank-1 b1 rows open each h psum group; no data deps
                    # beyond b1, so these fill PE gaps while DVE/ACT drain
                    # the previous expert
                    hp = {}
                    for (tag, ofs, n, nb) in H_TILES:
                        pt = ps_h.tile([T, n], f32, tag=tag, bufs=nb,
                                       name=f"h_{tag}_{e}")
                        hp[tag] = pt
                        b1s = b1_sb[0:1, 2 * I * e + ofs:2 * I * e + ofs + n]
                        nc.tensor.matmul(pt, ones_hf, b1s,
                                         start=True, stop=False)
                    return hp

                def emit_h_mm(e, hp):
                    w1_t = w1_tiles[e]
                    for d0 in (0, 2):
                        lhs = normed_f8[:, d0:d0 + 2, :]
                        for (tag, ofs, n, _) in H_TILES:
                            nc.tensor.matmul(
                                hp[tag], lhs,
                                w1_t[:, d0:d0 + 2, ofs:ofs + n],
                                perf_mode=DR, start=False, stop=False)
                    lhs = normed_f8[:, 4, :]
                    for (tag, ofs, n, _) in H_TILES:
                        nc.tensor.matmul(hp[tag], lhs,
                                         w1_t[:, 4, ofs:ofs + n],
                                         start=False, stop=True)

                def emit_act(e, hp):
                    # hact = (BETA/S1)*A * gm*sigmoid(BETA*gm/S1) * (l2/S1+1)
                    #      = BETA*A*hact_true
                    hact_b = hpool.tile([T, 512], f16, tag="hact_b",
                                        name=f"hab{e}")
                    hact_s = hpool.tile([T, 128], f16, tag="hact_s",
                                        name=f"has{e}")
                    for (n, gl, ln, ha) in (
                        (512, hp["hgb"], hp["hlb"], hact_b),
                        (128, hp["hsm"][:, 0:128], hp["hsm"][:, 128:256],
                         hact_s),
                    ):
                        gm = hpool.tile([T, n], f32, tag=f"gm{n}",
                                        name=f"gm{n}_{e}")
                        nc.vector.tensor_scalar(gm, gl, LIMIT * S1, None,
                                                op0=OP.min)
                        sg = hpool.tile([T, n], f32, tag=f"sg{n}",
                                        name=f"sg{n}_{e}")
                        nc.scalar.activation(sg, gm, AF.Sigmoid,
                                             scale=BETA / S1)
                        p_ = hpool.tile([T, n], f32, tag=f"p{n}",
                                        name=f"p{n}_{e}")
                        nc.gpsimd.tensor_mul(p_, gm, sg)
                        l2 = hpool.tile([T, n], f32, tag=f"l{n}",
                                        name=f"l{n}_{e}")
                        nc.vector.tensor_scalar(l2, ln, LIMIT * S1,
                                                -LIMIT * S1,
                                                op0=OP.min, op1=OP.max)
                        l3 = hpool.tile([T, n], f32, tag=f"l3{n}",
                                        name=f"l3{n}_{e}")
                        nc.scalar.activation(l3, l2, AF.Identity,
                                             bias=A_bb[:, e:e + 1],
                                             scale=A_sb[:, e:e + 1])
                        nc.gpsimd.tensor_mul(ha, p_, l3)
                    return hact_b, hact_s

                def emit_mm2(e, hact_b, hact_s, last):
                    w2_t = w2_tiles[e]
                    hT = hpool.tile([128, NCH, T], f8, tag="hT",
                                    name=f"hT{e}")
                    for c in range(NCH):
                        src = (hact_b[:, 128 * c:128 * (c + 1)]
                               if c < 4 else hact_s)
                        pt = ps_tr.tile([128, T], f16, tag="tr",
                                        name=f"tr{e}_{c}")
                        nc.tensor.transpose(pt, src, id_hf)
                        nc.vector.tensor_copy(hT[:, c, :], pt)
                    for c0 in (0, 2):
                        lhs = hT[:, c0:c0 + 2, :]
                        nc.tensor.matmul(o1, lhs,
                                         w2_t[:, c0:c0 + 2, 0:512],
                                         perf_mode=DR,
                                         start=False, stop=False)
                        nc.tensor.matmul(o2, lhs,
                                         w2_t[:, c0:c0 + 2, 512:640],
                                         perf_mode=DR,
                                         start=False, stop=False)
                    nc.tensor.matmul(o1, hT[:, 4, :], w2_t[:, 4, 0:512],
                                     start=False, stop=last)
                    nc.tensor.matmul(o2, hT[:, 4, :], w2_t[:, 4, 512:640],
                                     start=False, stop=last)

                hp = emit_h_bias(0)
                emit_h_mm(0, hp)
                # A-weighted b2 bias opens the o1/o2 accumulation groups
                ps_a = ps_tr.tile([K, T], f16, tag="tr", name="tra")
                nc.tensor.transpose(ps_a, A_hf, id_hf)
                a4t = small.tile([K, T], f16, tag="a4t")
                nc.scalar.activation(a4t, ps_a, AF.Identity)
                nc.tensor.matmul(o1, a4t, b2_t[:, 0:512],
                                 start=True, stop=False)
                nc.tensor.matmul(o2, a4t, b2_t[:, 512:640],
                                 start=True, stop=False)

                ha = emit_act(0, hp)
                for e in range(1, EPC):
                    hp_next = emit_h_bias(e)
                    emit_mm2(e - 1, *ha, last=False)
                    emit_h_mm(e, hp_next)
                    ha = emit_act(e, hp_next)
                emit_mm2(EPC - 1, *ha, last=True)

                acc = consts.tile([T, D], f32)
                nc.scalar.activation(acc[:, 0:512], o1, AF.Identity,
                                     scale=1.0 / (S2 * BETA))
                nc.scalar.activation(acc[:, 512:640], o2, AF.Identity,
                                     scale=1.0 / (S2 * BETA))

            nc.scalar.dma_start(out=dout.ap(), in_=acc)

    nc.finalize()
    return nc


def _get_nc():
    global _NC
    if _NC is None:
        _ensure_ntff_hook()
        _NC = _build()
    return _NC


# w1/b1 column permutation: [glu 0:512 | lin 0:512 | glu 512:640 | lin ...]
_HPERM = np.r_[0:512, 640:1152, 512:640, 1152:1280]


def _prep_weights(w1, b1, w2, b2):
    """Host-side scale + fp8 cast + tiling. Returns full-E arrays."""
    import ml_dtypes
    f8 = ml_dtypes.float8_e4m3
    w1q = (np.asarray(w1, np.float32)[:, :, _HPERM] * S1).astype(f8)
    b1q = (np.asarray(b1, np.float32)[:, _HPERM] * S1).astype(np.float16)
    w2q = (np.asarray(w2, np.float32) * S2).astype(f8)
    b2q = (np.asarray(b2, np.float32) * (S2 * BETA)).astype(np.float16)
    return w1q, b1q, w2q, b2q


def _ptile(a, nch):
    """(rows, F) -> (128, nch, F): partition p holds row c*128+p."""
    rows, F = a.shape
    return np.ascontiguousarray(
        a.reshape(nch, 128, F).transpose(1, 0, 2))


def kernel(**inputs):
    global LAST_EXEC_NS
    nc = _get_nc()
    from concourse.bass_utils import run_bass_kernel_spmd

    x = np.ascontiguousarray(np.asarray(inputs["x"], dtype=np.float32))
    norm_w = np.asarray(inputs["norm_w"], np.float32)
    gate_w = np.asarray(inputs["gate_w"], np.float32)
    gate_b = np.ascontiguousarray(np.asarray(inputs["gate_b"], np.float32))
    w1q, b1q, w2q, b2q = _prep_weights(inputs["w1"], inputs["b1"],
                                       inputs["w2"], inputs["b2"])

    x2 = np.ascontiguousarray(x[0, :, 0, :])  # (D, T)
    xt = _ptile(x2, NCH)
    nwt = np.ascontiguousarray(norm_w.reshape(NCH, 128).T)
    in_maps = []
    for c in range(NCORES):
        lo, hi = EPC * c, EPC * (c + 1)
        perm = np.r_[lo:hi, 0:lo, hi:E]
        # (EPC, D, F) -> (128, EPC, NCH, F): partition p holds row c*128+p
        w1c = np.ascontiguousarray(
            w1q[lo:hi].reshape(EPC, NCH, 128, 2 * I).transpose(2, 0, 1, 3))
        w2c = np.ascontiguousarray(
            w2q[lo:hi].reshape(EPC, NCH, 128, D).transpose(2, 0, 1, 3))
        in_maps.append({
            "x": xt,
            "norm_w": nwt,
            "gate_wt": _ptile(
                np.ascontiguousarray(gate_w[perm].T.astype(np.float16)), NCH),
            "gate_b": np.ascontiguousarray(gate_b[perm]),
            "w1": w1c,
            "b1": np.ascontiguousarray(b1q[lo:hi]),
            "w2": w2c,
            "b2": np.ascontiguousarray(b2q[lo:hi]),
        })

    res = run_bass_kernel_spmd(nc, in_maps, core_ids=list(range(NCORES)),
                               trace=TRACE, tmpdir=PROF_DIR)
    LAST_EXEC_NS = res.exec_time_ns
    total = np.sum([r["out"] for r in res.results], axis=0)  # (T, D)
    return (x + total.T[None, :, None, :]).astype(np.float32)
